# revision 1
# baseline (speedup 1.0000x reference)
"""Trainium2 Bass kernel for ContextQueryAttention (BiDAF-style trilinear
attention). Data-parallel over batch across 8 NeuronCores (4 batches/core).

Per batch (c=1024 context rows, q=128 query rows, h=256 hidden):
  S[c,q]   = ctx@cw + (qry@qw)^T + (ctx*cqw)@qry^T + bias
  S_bar    = softmax_c(S); S_bar_bar = softmax_q(S)
  A        = S @ qry
  B        = S_bar @ (S_bar_bar^T @ ctx)
  out      = concat([ctx, A, ctx*A, ctx*B], -1)

Layout strategy: the h-contraction operands (ctx^T, qry^T) are staged
host-side during sharding (pre-rounded to fp32r), so the PE runs only true
matmuls plus 8 small exp-transposes per batch. S^T [q,c] carries s0/bias via
a K=1 augmented matmul; s1 rides the exp bias (softmax-invariant terms need
only appear where raw S is consumed). One exp pass (ACT, fused row-sums)
serves both softmaxes: softmax_q normalizers are folded into the T-matmul's
ctx operand, softmax_c normalizers into T itself, so the unnormalized exp
matrix is the lhsT of both the T and B matmuls.
"""

import numpy as np

B, C, Q, H = 32, 1024, 128, 256
N_CORES = 8
BPC = B // N_CORES  # batches per core
P = 128
HC = H // P  # h chunks of 128
CT = C // P  # c tiles of 128
CCH = 512  # S^T free-dim chunk (1 PSUM bank of fp32)
NCC = C // CCH

_NC_CACHE = {}


def _round_f32r(a):
    """Round-to-nearest f32 -> fp32r (e8m11) so the PE's fp32r path sees
    pre-rounded values (it consumes only the top 20 bits)."""
    u = a.view(np.uint32)
    return ((u + 0x800) & np.uint32(0xFFFFF000)).view(np.float32)


def _build_kernel():
    import concourse.bacc as bacc
    import concourse.bass as bass
    import concourse.tile as tile
    from concourse import mybir
    from concourse.masks import make_identity

    f32 = mybir.dt.float32
    f32r = mybir.dt.float32r
    bf16 = mybir.dt.bfloat16
    AF = mybir.ActivationFunctionType
    AX = mybir.AxisListType
    ALU = mybir.AluOpType

    nc = bacc.Bacc(trn_type="TRN2", target_bir_lowering=False, debug=False)
    ctx_d = nc.dram_tensor("ctx", [BPC, C, H], f32, kind="ExternalInput").ap()
    ctxT_d = nc.dram_tensor("ctxT", [BPC, H, C], bf16, kind="ExternalInput").ap()
    qry_d = nc.dram_tensor("qry", [BPC, Q, H], f32, kind="ExternalInput").ap()
    # host-staged in SBUF layout [p, j, b, q] so the DMA is a flat copy
    qryT_d = nc.dram_tensor("qryT", [P, HC * BPC * Q], f32, kind="ExternalInput").ap()
    cw_d = nc.dram_tensor("cw", [H], f32, kind="ExternalInput").ap()
    qw_d = nc.dram_tensor("qw", [H], f32, kind="ExternalInput").ap()
    cqw_d = nc.dram_tensor("cqw", [H], f32, kind="ExternalInput").ap()
    bias_d = nc.dram_tensor("bias", [1, 1], f32, kind="ExternalInput").ap()
    out_d = nc.dram_tensor("out", [BPC, C, 4 * H], f32, kind="ExternalOutput").ap()

    from contextlib import ExitStack

    with tile.TileContext(nc) as tc, ExitStack() as es:
        consts = es.enter_context(tc.tile_pool(name="consts", bufs=1))
        p_ctx = es.enter_context(tc.tile_pool(name="p_ctx", bufs=2))
        p_ctxT = es.enter_context(tc.tile_pool(name="p_ctxT", bufs=2))
        p_q = es.enter_context(tc.tile_pool(name="p_q", bufs=2))
        p_big = es.enter_context(tc.tile_pool(name="p_big", bufs=2))
        p_med = es.enter_context(tc.tile_pool(name="p_med", bufs=2))
        p_aug = es.enter_context(tc.tile_pool(name="p_aug", bufs=2))
        p_out = es.enter_context(tc.tile_pool(name="p_out", bufs=2))
        pp_tr = es.enter_context(tc.tile_pool(name="pp_tr", bufs=2, space="PSUM"))
        pp_st = es.enter_context(tc.tile_pool(name="pp_st", bufs=2, space="PSUM"))
        pp_mm = es.enter_context(tc.tile_pool(name="pp_mm", bufs=2, space="PSUM"))
        pp_t = es.enter_context(tc.tile_pool(name="pp_t", bufs=2, space="PSUM"))

        identity = consts.tile([P, P], bf16)
        make_identity(nc, identity)
        cw_col = consts.tile([P, HC], f32)
        nc.sync.dma_start(out=cw_col, in_=cw_d.rearrange("(j p) -> p j", p=P))
        cq_col = consts.tile([P, HC], f32)
        nc.sync.dma_start(out=cq_col, in_=cqw_d.rearrange("(j p) -> p j", p=P))
        bias_sb = consts.tile([1, 1], f32)
        nc.sync.dma_start(out=bias_sb, in_=bias_d)
        ones_c_f = consts.tile([1, C], f32)
        nc.vector.memset(ones_c_f, 1.0)
        ones_q = consts.tile([1, Q], bf16)
        nc.vector.tensor_copy(ones_q, ones_c_f[:, 0:Q])
        cw_colr = consts.tile([P, HC], bf16)
        nc.vector.tensor_copy(cw_colr, cw_col)

        ones_c = consts.tile([1, C], bf16)
        nc.vector.tensor_copy(ones_c, ones_c_f)
        qw_col = consts.tile([P, HC], f32)
        nc.sync.dma_start(out=qw_col, in_=qw_d.rearrange("(j p) -> p j", p=P))
        qw_colr = consts.tile([P, HC], bf16)
        nc.vector.tensor_copy(qw_colr, qw_col)

        # all 4 batches of query in one DMA: [q, b, h]
        q_all = consts.tile([P, BPC, H], f32)
        nc.sync.dma_start(out=q_all, in_=qry_d.rearrange("b q h -> q b h"))
        q_all_r = consts.tile([P, BPC, H], bf16)
        nc.vector.tensor_copy(q_all_r, q_all)
        # all 4 batches of qry^T in one DMA: [p, j, b, q]
        qt_all = consts.tile([P, HC, BPC, Q], f32)
        nc.sync.dma_start(out=qt_all.rearrange("p j b q -> p (j b q)"), in_=qryT_d)

        for b in range(BPC):
            qry = q_all[:, b, :]

            ctx_nat = p_ctx.tile([P, CT, H], f32, tag="ctx_nat")
            nc.sync.dma_start(
                out=ctx_nat, in_=ctx_d[b].rearrange("(t p) h -> p t h", p=P)
            )
            ctxT = p_ctxT.tile([P, HC, C], bf16, tag="ctxT")
            nc.sync.dma_start(
                out=ctxT, in_=ctxT_d[b].rearrange("(j p) c -> p j c", p=P)
            )

            # qt_cq[h, q] = qry^T * cq_weight (also the fp32r cast)
            qt_cq = p_q.tile([P, HC, Q], bf16, tag="qt_cq")
            for j in range(HC):
                nc.vector.tensor_scalar_mul(
                    qt_cq[:, j], qt_all[:, j, b, :], cq_col[:, j : j + 1]
                )

            # s1 row = (qry @ qw)^T as [1, q] (v1-style PE path)
            qt_raw = p_q.tile([P, HC, Q], bf16, tag="qt_raw")
            for j in range(HC):
                nc.vector.tensor_copy(qt_raw[:, j], qt_all[:, j, b, :])
            s1p = pp_st.tile([1, Q], f32, tag="stp")
            for j in range(HC):
                nc.tensor.matmul(
                    s1p,
                    lhsT=qw_colr[:, j : j + 1],
                    rhs=qt_raw[:, j],
                    start=(j == 0),
                    stop=(j == HC - 1),
                )
            s1_row = p_aug.tile([1, Q], bf16, tag="s1_row")
            nc.scalar.copy(s1_row, s1p)

            # ---- s0 row = ctx @ cw (+bias) as [1, c] ----
            s0_row = p_aug.tile([1, C], bf16, tag="s0_row")
            for cc in range(NCC):
                s0p = pp_st.tile([1, CCH], f32, tag="stp")
                for j in range(HC):
                    nc.tensor.matmul(
                        s0p,
                        lhsT=cw_colr[:, j : j + 1],
                        rhs=ctxT[:, j, cc * CCH : (cc + 1) * CCH],
                        start=(j == 0),
                        stop=(j == HC - 1),
                    )
                nc.scalar.activation(
                    s0_row[0:1, cc * CCH : (cc + 1) * CCH],
                    s0p,
                    AF.Identity,
                    bias=bias_sb[0:1, :],
                    scale=1.0,
                )

            # ---- S^T [q, c] = qt_cq.T @ ctxT + ones_q x s0_row; exp ----
            # (s1 rides the exp bias / the raw-S drain; it is softmax-invariant
            # along c and handled per-partition here)
            e_t = p_big.tile([P, C], bf16, tag="e_t")
            st_raw = p_big.tile([P, C], bf16, tag="st_raw")
            rsum = p_aug.tile([P, NCC], f32, tag="rsum")
            for cc in range(NCC):
                sl = slice(cc * CCH, (cc + 1) * CCH)
                stp = pp_st.tile([P, CCH], f32, tag="stp")
                for j in range(HC):
                    nc.tensor.matmul(
                        stp,
                        lhsT=qt_cq[:, j],
                        rhs=ctxT[:, j, sl],
                        start=(j == 0),
                        stop=False,
                    )
                nc.tensor.matmul(
                    stp,
                    lhsT=s1_row,
                    rhs=ones_c[:, sl],
                    start=False,
                    stop=False,
                )
                nc.tensor.matmul(
                    stp,
                    lhsT=ones_q,
                    rhs=s0_row[:, sl],
                    start=False,
                    stop=True,
                )
                nc.scalar.activation(
                    e_t[:, sl], stp, AF.Exp, accum_out=rsum[:, cc : cc + 1]
                )
                nc.vector.tensor_copy(st_raw[:, sl], stp)

            # softmax_c denominators: rq[q] = 1 / sum_c exp
            zq = p_aug.tile([P, 1], f32, tag="zq")
            nc.vector.reduce_sum(zq, rsum, axis=AX.X)
            rq = p_aug.tile([P, 1], f32, tag="rq")
            nc.vector.reciprocal(rq, zq)

            # ---- E-transpose per c-tile; softmax_q normalizers into ctx ----
            e_sb = p_med.tile([P, CT, P], bf16, tag="e_sb")
            ctx_s = p_med.tile([P, CT, H], bf16, tag="ctx_s")
            zc = p_aug.tile([P, CT], f32, tag="zc")
            rc = p_aug.tile([P, CT], f32, tag="rc")
            for t in range(CT):
                pe_ = pp_tr.tile([P, P], bf16, tag="tr")
                nc.tensor.transpose(pe_, e_t[:, t * P : (t + 1) * P], identity)
                nc.vector.reduce_sum(zc[:, t : t + 1], pe_, axis=AX.X)
                nc.vector.reciprocal(rc[:, t : t + 1], zc[:, t : t + 1])
                nc.scalar.copy(e_sb[:, t, :], pe_)
                nc.vector.tensor_scalar_mul(
                    ctx_s[:, t, :], ctx_nat[:, t, :], rc[:, t : t + 1]
                )

            # ---- T = S_bar_bar^T @ ctx as one tight accumulation group ----
            t_acc = pp_t.tile([P, H], f32, tag="t_acc")
            for t in range(CT):
                nc.tensor.matmul(
                    t_acc,
                    lhsT=e_sb[:, t, :],
                    rhs=ctx_s[:, t, :],
                    start=(t == 0),
                    stop=(t == CT - 1),
                )
            # fold softmax_c normalizer into T
            ts = p_med.tile([P, H], bf16, tag="ts")
            nc.vector.tensor_scalar_mul(ts, t_acc, rq)

            # ---- A and B per c-tile; assemble output channels ----
            out_t = p_out.tile([P, CT, 3 * H], f32, tag="out_t")
            for t in range(CT):
                sl = slice(t * P, (t + 1) * P)
                pa = pp_mm.tile([P, H], f32, tag="ab")
                nc.tensor.matmul(
                    pa,
                    lhsT=st_raw[:, sl],
                    rhs=q_all_r[:, b, :],
                    start=True,
                    stop=True,
                )
                nc.scalar.copy(out_t[:, t, 0:H], pa)
                nc.vector.tensor_mul(out_t[:, t, H : 2 * H], ctx_nat[:, t, :], pa)
                pb = pp_mm.tile([P, H], f32, tag="ab")
                nc.tensor.matmul(
                    pb,
                    lhsT=e_t[:, sl],
                    rhs=ts,
                    start=True,
                    stop=True,
                )
                nc.vector.tensor_mul(out_t[:, t, 2 * H : 3 * H], ctx_nat[:, t, :], pb)

            # ---- stores ----
            nc.sync.dma_start(
                out=out_d[b, :, 0:H].rearrange("(t p) h -> p t h", p=P), in_=ctx_nat
            )
            nc.sync.dma_start(
                out=out_d[b, :, H : 4 * H].rearrange("(t p) h -> p t h", p=P),
                in_=out_t,
            )

    nc.compile()
    return nc


def _get_nc():
    if "nc" not in _NC_CACHE:
        _NC_CACHE["nc"] = _build_kernel()
    return _NC_CACHE["nc"]


def make_in_maps(context, query, c_weight, q_weight, cq_weight, bias):
    context = np.ascontiguousarray(np.asarray(context, dtype=np.float32))
    query = np.ascontiguousarray(np.asarray(query, dtype=np.float32))
    cw = np.asarray(c_weight, dtype=np.float32).reshape(H).copy()
    qw = np.asarray(q_weight, dtype=np.float32).reshape(H).copy()
    cqw = np.asarray(cq_weight, dtype=np.float32).reshape(H).copy()
    bs = np.asarray(bias, dtype=np.float32).reshape(1, 1).copy()

    in_maps = []
    for i in range(N_CORES):
        sl = slice(i * BPC, (i + 1) * BPC)
        ctx_i = np.ascontiguousarray(context[sl])
        qry_i = np.ascontiguousarray(query[sl])
        import ml_dtypes

        ctxT_i = np.ascontiguousarray(ctx_i.transpose(0, 2, 1)).astype(ml_dtypes.bfloat16)
        # [BPC, H, Q] -> [P, HC, BPC, Q] (SBUF layout) -> flat [P, HC*BPC*Q]
        qryT_i = np.ascontiguousarray(
            qry_i.transpose(0, 2, 1)
            .reshape(BPC, HC, P, Q)
            .transpose(2, 1, 0, 3)
            .reshape(P, HC * BPC * Q)
        )
        in_maps.append(
            {
                "ctx": ctx_i,
                "ctxT": ctxT_i,
                "qry": qry_i,
                "qryT": qryT_i,
                "cw": cw,
                "qw": qw,
                "cqw": cqw,
                "bias": bs,
            }
        )
    return in_maps


def kernel(context, query, c_mask, q_mask, c_weight, q_weight, cq_weight, bias):
    from concourse import bass_utils

    nc = _get_nc()
    in_maps = make_in_maps(context, query, c_weight, q_weight, cq_weight, bias)
    res = bass_utils.run_bass_kernel_spmd(nc, in_maps, core_ids=list(range(N_CORES)))
    return np.concatenate([res.results[i]["out"] for i in range(N_CORES)], axis=0)



# revision 4
# speedup vs baseline: 1.1313x; 1.1313x over previous
"""Trainium2 Bass kernel for ContextQueryAttention (BiDAF-style trilinear
attention). Data-parallel over batch across 8 NeuronCores (4 batches/core).

Per batch (c=1024 context rows, q=128 query rows, h=256 hidden):
  S[c,q]   = ctx@cw + (qry@qw)^T + (ctx*cqw)@qry^T + bias
  S_bar    = softmax_c(S); S_bar_bar = softmax_q(S)
  A        = S @ qry
  B        = S_bar @ (S_bar_bar^T @ ctx)
  out      = concat([ctx, A, ctx*A, ctx*B], -1)

Layout/fusion strategy:
  - All heavy tensors are host-staged to bf16 in partition-major layouts so
    every DMA is a flat 4KB-per-partition copy.
  - s0 = ctx@cw rides the main S^T matmul for free: the query-side operand is
    qt_aug[h,q] = qryT[h,q]*cqw[h] + cw[h], so sum_h ctx[c,h]*qt_aug[h,q]
    = s2[c,q] + s0[c].
  - s1+bias is per-partition in the S^T [q,c] layout: it rides the Exp
    activation's bias operand (e_t) and a tensor_scalar add (st_raw).
  - Both softmax denominators come from accumulator side-outputs: zq from the
    Exp accum, zc from the transpose-copy accum. Normalizers are folded into
    the small operands (e_ss = e_sb*rc, ts = T*rq).
  - Output is assembled per 128-row tile into a [128, 4H] fp32 tile
    (chunk0=ctx on GpSimd, chunk1=A on ScalarE, chunk2/3=ctx*A/ctx*B on
    VectorE) and streamed out with per-tile DMAs (4KB rows).
"""

import numpy as np

B, C, Q, H = 32, 1024, 128, 256
N_CORES = 8
BPC = B // N_CORES  # batches per core
P = 128
HC = H // P  # h chunks of 128
CT = C // P  # c tiles of 128
CCH = 512  # S^T free-dim chunk (1 PSUM bank of fp32)
NCC = C // CCH

_NC_CACHE = {}


def _build_kernel(compile=True):
    import concourse.bacc as bacc
    import concourse.tile as tile
    from concourse import mybir
    from concourse.masks import make_identity
    from contextlib import ExitStack

    f32 = mybir.dt.float32
    bf16 = mybir.dt.bfloat16
    AF = mybir.ActivationFunctionType
    AX = mybir.AxisListType
    ALU = mybir.AluOpType

    nc = bacc.Bacc(trn_type="TRN2", target_bir_lowering=False, debug=False)
    ctx_d = nc.dram_tensor("ctx", [BPC, P, CT * H], bf16, kind="ExternalInput").ap()
    ctxT_d = nc.dram_tensor("ctxT", [BPC, P, HC * C], bf16, kind="ExternalInput").ap()
    q_d = nc.dram_tensor("qn", [P, BPC * H], bf16, kind="ExternalInput").ap()
    qt_d = nc.dram_tensor("qt", [P, HC * BPC * Q], bf16, kind="ExternalInput").ap()
    cw_d = nc.dram_tensor("cw", [P, HC], f32, kind="ExternalInput").ap()
    cq_d = nc.dram_tensor("cq", [P, HC], f32, kind="ExternalInput").ap()
    qw_d = nc.dram_tensor("qw", [P, HC], bf16, kind="ExternalInput").ap()
    bias_d = nc.dram_tensor("bias", [P, 1], f32, kind="ExternalInput").ap()
    out_d = nc.dram_tensor("out", [BPC, C, 4 * H], f32, kind="ExternalOutput").ap()

    with tile.TileContext(nc) as tc, ExitStack() as es:
        consts = es.enter_context(tc.tile_pool(name="consts", bufs=1))
        p_ctx = es.enter_context(tc.tile_pool(name="p_ctx", bufs=2))
        p_ctxT = es.enter_context(tc.tile_pool(name="p_ctxT", bufs=2))
        p_big = es.enter_context(tc.tile_pool(name="p_big", bufs=2))
        p_med = es.enter_context(tc.tile_pool(name="p_med", bufs=2))
        p_small = es.enter_context(tc.tile_pool(name="p_small", bufs=2))
        p_out = es.enter_context(tc.tile_pool(name="p_out", bufs=6))
        # PSUM is 8 banks, one bank per pool buf: stp 2 + pab 2 + t_acc 2 +
        # tr 2 (s1p shares the tr tag's rotation).
        pp_st = es.enter_context(tc.tile_pool(name="pp_st", bufs=2, space="PSUM"))
        pp_ab = es.enter_context(tc.tile_pool(name="pp_ab", bufs=2, space="PSUM"))
        pp_t = es.enter_context(tc.tile_pool(name="pp_t", bufs=2, space="PSUM"))
        pp_tr = es.enter_context(tc.tile_pool(name="pp_tr", bufs=2, space="PSUM"))

        identity = consts.tile([P, P], bf16)
        make_identity(nc, identity)
        cw_col = consts.tile([P, HC], f32)
        nc.sync.dma_start(out=cw_col, in_=cw_d)
        cq_col = consts.tile([P, HC], f32)
        nc.sync.dma_start(out=cq_col, in_=cq_d)
        qw_col = consts.tile([P, HC], bf16)
        nc.sync.dma_start(out=qw_col, in_=qw_d)
        bias_col = consts.tile([P, 1], f32)
        nc.sync.dma_start(out=bias_col, in_=bias_d)
        q_all = consts.tile([P, BPC * H], bf16)
        nc.sync.dma_start(out=q_all, in_=q_d)
        qt_all = consts.tile([P, HC * BPC * Q], bf16)
        nc.sync.dma_start(out=qt_all, in_=qt_d)

        for b in range(BPC):
            ctx = p_ctx.tile([P, CT * H], bf16, tag="ctx")
            nc.sync.dma_start(out=ctx, in_=ctx_d[b])
            ctxT = p_ctxT.tile([P, HC * C], bf16, tag="ctxT")
            nc.sync.dma_start(out=ctxT, in_=ctxT_d[b])

            # qt_aug[h,q] = qryT*cqw + cw  (carries s0 through the S matmuls)
            qt_aug = p_small.tile([P, HC * Q], bf16, tag="qt_aug")
            for j in range(HC):
                nc.vector.tensor_scalar(
                    qt_aug[:, j * Q : (j + 1) * Q],
                    qt_all[:, (j * BPC + b) * Q : (j * BPC + b + 1) * Q],
                    cq_col[:, j : j + 1],
                    cw_col[:, j : j + 1],
                    ALU.mult,
                    ALU.add,
                )
            # s1 column (+ bias): s1b[q] = qry[q,:]@qw + bias
            s1p = pp_tr.tile([P, 1], f32, tag="tr")
            for j in range(HC):
                nc.tensor.matmul(
                    s1p,
                    lhsT=qt_all[:, (j * BPC + b) * Q : (j * BPC + b + 1) * Q],
                    rhs=qw_col[:, j : j + 1],
                    start=(j == 0),
                    stop=(j == HC - 1),
                )
            s1b = p_small.tile([P, 1], f32, tag="s1b")
            nc.vector.tensor_scalar_add(s1b, s1p, bias_col)

            # ---- S^T [q,c] chunks; e_t = exp(S^T) (zq via accum); raw S^T ----
            e_t = p_big.tile([P, C], bf16, tag="e_t")
            st_raw = p_big.tile([P, C], bf16, tag="st_raw")
            rsum = p_small.tile([P, NCC], f32, tag="rsum")
            for cc in range(NCC):
                sl = slice(cc * CCH, (cc + 1) * CCH)
                stp = pp_st.tile([P, CCH], f32, tag="stp")
                for j in range(HC):
                    nc.tensor.matmul(
                        stp,
                        lhsT=qt_aug[:, j * Q : (j + 1) * Q],
                        rhs=ctxT[:, j * C + cc * CCH : j * C + (cc + 1) * CCH],
                        start=(j == 0),
                        stop=(j == HC - 1),
                    )
                nc.scalar.activation(
                    e_t[:, sl],
                    stp,
                    AF.Exp,
                    bias=s1b,
                    scale=1.0,
                    accum_out=rsum[:, cc : cc + 1],
                )
                nc.vector.tensor_scalar_add(st_raw[:, sl], stp, s1b)
            zq = p_small.tile([P, 1], f32, tag="zq")
            nc.vector.reduce_sum(zq, rsum, axis=AX.X)
            rq = p_small.tile([P, 1], f32, tag="rq")
            nc.vector.reciprocal(rq, zq)

            # ---- transpose e_t per c-tile; zc via copy accum; e_ss = e*rc ----
            e_sb = p_med.tile([P, CT * P], bf16, tag="e_sb")
            e_ss = p_med.tile([P, CT * P], bf16, tag="e_ss")
            zc = p_small.tile([P, CT], f32, tag="zc")
            rc = p_small.tile([P, CT], f32, tag="rc")
            for t in range(CT):
                pe_ = pp_tr.tile([P, P], bf16, tag="tr")
                nc.tensor.transpose(pe_, e_t[:, t * P : (t + 1) * P], identity)
                nc.scalar.activation(
                    e_sb[:, t * P : (t + 1) * P],
                    pe_,
                    AF.Identity,
                    accum_out=zc[:, t : t + 1],
                )
            nc.vector.reciprocal(rc, zc)
            for t in range(CT):
                nc.vector.tensor_scalar_mul(
                    e_ss[:, t * P : (t + 1) * P],
                    e_sb[:, t * P : (t + 1) * P],
                    rc[:, t : t + 1],
                )

            # ---- T = S_bar_bar^T @ ctx; ts = T * rq ----
            t_acc = pp_t.tile([P, H], f32, tag="t_acc")
            for t in range(CT):
                nc.tensor.matmul(
                    t_acc,
                    lhsT=e_ss[:, t * P : (t + 1) * P],
                    rhs=ctx[:, t * H : (t + 1) * H],
                    start=(t == 0),
                    stop=(t == CT - 1),
                )
            ts = p_small.tile([P, H], bf16, tag="ts")
            nc.vector.tensor_scalar_mul(ts, t_acc, rq)

            # ---- per c-tile: A & B matmuls, assemble [ctx|A|ctx*A|ctx*B] ----
            qb = q_all[:, b * H : (b + 1) * H]
            for t in range(CT):
                sl = slice(t * P, (t + 1) * P)
                hsl = slice(t * H, (t + 1) * H)
                pab = pp_ab.tile([P, 2 * H], f32, tag="ab")
                nc.tensor.matmul(
                    pab[:, 0:H], lhsT=st_raw[:, sl], rhs=qb, start=True, stop=True
                )
                nc.tensor.matmul(
                    pab[:, H : 2 * H], lhsT=e_t[:, sl], rhs=ts, start=True, stop=True
                )
                ot = p_out.tile([P, 4 * H], f32, tag="ot")
                nc.gpsimd.tensor_copy(ot[:, 0:H], ctx[:, hsl])
                nc.scalar.copy(ot[:, H : 2 * H], pab[:, 0:H])
                nc.vector.tensor_mul(ot[:, 2 * H : 3 * H], ctx[:, hsl], pab[:, 0:H])
                nc.vector.tensor_mul(
                    ot[:, 3 * H : 4 * H], ctx[:, hsl], pab[:, H : 2 * H]
                )
                nc.sync.dma_start(out=out_d[b, t * P : (t + 1) * P, :], in_=ot)

    if compile:
        nc.compile()
    return nc


def _get_nc():
    if "nc" not in _NC_CACHE:
        _NC_CACHE["nc"] = _build_kernel()
    return _NC_CACHE["nc"]


def make_in_maps(context, query, c_weight, q_weight, cq_weight, bias):
    import ml_dtypes

    bf16 = ml_dtypes.bfloat16
    context = np.ascontiguousarray(np.asarray(context, dtype=np.float32))
    query = np.ascontiguousarray(np.asarray(query, dtype=np.float32))
    cw = np.asarray(c_weight, dtype=np.float32).reshape(H)
    qw = np.asarray(q_weight, dtype=np.float32).reshape(H)
    cqw = np.asarray(cq_weight, dtype=np.float32).reshape(H)
    bs = float(np.asarray(bias, dtype=np.float32).reshape(()))

    cw_col = np.ascontiguousarray(cw.reshape(HC, P).T)
    cq_col = np.ascontiguousarray(cqw.reshape(HC, P).T)
    qw_col = np.ascontiguousarray(qw.reshape(HC, P).T).astype(bf16)
    bias_col = np.full((P, 1), bs, dtype=np.float32)

    in_maps = []
    for i in range(N_CORES):
        sl = slice(i * BPC, (i + 1) * BPC)
        ctx_i = context[sl]
        qry_i = query[sl]
        # natural, partition-major: [b, p, t*h] with row c = t*P + p
        ctx_n = np.ascontiguousarray(
            ctx_i.reshape(BPC, CT, P, H).transpose(0, 2, 1, 3).reshape(BPC, P, CT * H)
        ).astype(bf16)
        # transposed, partition-major: [b, p, j*c] with col h = j*P + p
        ctxT_i = np.ascontiguousarray(
            ctx_i.transpose(0, 2, 1)
            .reshape(BPC, HC, P, C)
            .transpose(0, 2, 1, 3)
            .reshape(BPC, P, HC * C)
        ).astype(bf16)
        # qry natural on q-partitions: [p=q, b*h]
        q_n = np.ascontiguousarray(qry_i.transpose(1, 0, 2).reshape(P, BPC * H)).astype(
            bf16
        )
        # qryT on h-partitions: [p, (j b q)]
        qt_i = np.ascontiguousarray(
            qry_i.transpose(0, 2, 1)
            .reshape(BPC, HC, P, Q)
            .transpose(2, 1, 0, 3)
            .reshape(P, HC * BPC * Q)
        ).astype(bf16)
        in_maps.append(
            {
                "ctx": ctx_n,
                "ctxT": ctxT_i,
                "qn": q_n,
                "qt": qt_i,
                "cw": cw_col,
                "cq": cq_col,
                "qw": qw_col,
                "bias": bias_col,
            }
        )
    return in_maps


def kernel(context, query, c_mask, q_mask, c_weight, q_weight, cq_weight, bias):
    from concourse import bass_utils

    nc = _get_nc()
    in_maps = make_in_maps(context, query, c_weight, q_weight, cq_weight, bias)
    res = bass_utils.run_bass_kernel_spmd(nc, in_maps, core_ids=list(range(N_CORES)))
    return np.concatenate([res.results[i]["out"] for i in range(N_CORES)], axis=0)
